# revision 29
# baseline (speedup 1.0000x reference)
"""AttnBlock (GroupNorm -> QKV 1x1 -> attention over H -> proj -> residual)
for B=8, C=512, H=2048 on 8 trn2 NeuronCores, data-parallel over batch.

Each core runs the full block for one batch element. All four heavy GEMMs
run as fp8e4 (TRN e4m3, max 240) DoubleRow matmuls: two 128-deep K-chunks
packed per instruction (lhsT [K,2,M], rhs [K,2,N]) at 2x the fp16 row
rate, with f32 PSUM accumulation. The folded weights MT = Wk^T Wq and
WPV^T = (Wp Wv)^T are scaled by 16 on the host so their ~N(0, 1/512)
entries clear the e4m3 subnormal cutoff (2^-6) with a >2x headroom under
the 240 max on the GEMM outputs (|16 z|, |16 vp| <~ 130). The 16x scales
fold into the exp scale (SCL/16) and the denominator ones-matmul constant
(16.0). exp gets a -2 ln 2 bias so fp8 est tops out ~75 « 240; the shift
cancels exactly in the softmax ratio. CPU sim: ~1.0e-2 scale-relative
error vs the f32 reference (gate 2e-2; fp16 baseline was 3.8e-4).

Structure per core:
  A. x as fp16 over three parallel DMA queues (sync+scalar split the
     two stats-anchoring chunks; gpsimd alone carries consts + the fp8
     weight slabs, mt pairs first -- the z GEMM needs them earliest);
     group-norm stats on DVE bn_stats for 3 chunks + ACT accum_out
     (Copy/Square) for one; xq8 = fp8(a*x + b) quantize pass, sg-major,
     into DoubleRow pair tiles [P,2,H] (x stays resident in fp16 for
     the residual).
  B. z8 = fp8(16 * M @ hn) via 2 DoubleRow matmuls per [128,512] tile
     (M = Wk^T Wq folds the q and k GEMMs; scores are bilinear in hn).
  C. vT8 = fp8((hn^T (16 WPV^T))) with WPV = Wp @ Wv: since sum_j
     softmax = 1, the proj GEMM folds away entirely -- out =
     (WPV hn) @ probs^T / den + bp2 + x with bp2 = Wp bv + bp.
  D. per 512-wide i-tile: scores^T via 2 DoubleRow matmuls per j-chunk
     -> exp -> est8 (fp8 pair tiles, write-only for ACT); denominator
     = 8 more DoubleRow ones-matmuls on the PE itself (sums over
     partitions AND broadcasts to all 128; any DVE/Pool add tree over
     fp8 tiles measured 1.4-1.9us per add and stalled the PE 7-8us per
     i-tile); 1/(16 den) as exp(-ln) on ACT (DVE reciprocal is ~4us
     per [128,512] and sat on the PSUM-bank recycle path); output
     chunks = 8 DoubleRow matmuls, scaled by 1/(16 den) on DVE + bias
     + residual; final i-tile outputs split across the sync and scalar
     queues (both idle by then, fast drain).

Engine balance at ~116us wall (vs 174us fp16 baseline): PE ~80us busy
(352 matmuls, 216ns each at full clock, LDWEIGHTS fully hidden), ACT
~72us (64 exps at ~730ns dominate), DVE ~67us, head ~22us (startup +
DMA + serial stats chain), tail ~8us.

Known hazards on this stack (empirical): long PE warmup streams or a
heavily loaded gpsimd DMA queue trigger a chip-wide ~1.2x DVFS clock
demotion for the rest of the kernel; inserting extra PE/ACT ops inside
the stats chain (dummy table-load or warmup matmuls) hangs execution.

Walrus limits compute-engine instructions to one sync wait each, while Tile
emits all required waits on the first instruction that needs them. We route
every emitted op through a wrapper that can plant same-engine spare NoOps
directly before it; a post-scheduling pass moves excess waits onto the
spares, and a build fixpoint discovers where spares are needed.
"""

import sys

sys.path.insert(0, "/opt/trn_rl_repo")

import numpy as np

B, C, H = 8, 512, 2048
GROUPS = 32
EPS = 1e-6
P = 128
FD = 512
NCH = C // P  # 4 channel chunks
NPR = NCH // 2  # 2 DoubleRow channel-pair tiles
NJT = H // P  # 16 j-chunks
NJP = NJT // 2  # 8 DoubleRow j-pair tiles
NIT = H // FD  # 4 i-tiles
GPC = 8  # groups per 128-channel chunk (128/16)
CPG = C // GROUPS  # 16 channels per group
SCL = float(C) ** -0.5
SC_W = 16.0  # host-side scale on MT and WPV^T for fp8 range
EXP_BIAS = float(-2.0 * np.log(2.0))  # cancels in softmax; keeps est < 240

# instruction types never subject to the 1-wait redistribution
_EXEMPT = ("InstEventSemaphore", "InstNoOp")

_ENG_ATTR = {
    "EngineType.PE": "tensor",
    "EngineType.DVE": "vector",
    "EngineType.Activation": "scalar",
    "EngineType.Pool": "gpsimd",
    "EngineType.SP": "sync",
}


class _Emitter:
    """Emission wrapper: plants pinned spare NoOps before instructions that
    the previous build iteration found to carry >1 sync wait."""

    def __init__(self, nc, tile_mod, needed):
        self.nc = nc
        self.tile = tile_mod
        self.needed = needed
        self.spare_owner = {}  # spare inst name -> key
        self.inst_key = {}  # real inst name -> key
        self.counters = {}
        self.last = {}

    def __call__(self, engine, method, *args, **kw):
        eng = getattr(self.nc, engine)
        idx = self.counters.get(engine, 0)
        self.counters[engine] = idx + 1
        key = (engine, idx)
        nops = []
        for _ in range(self.needed.get(key, 0) if key[0] != "tail" else 0):
            nop = eng.nop(nofuse=True, hint="sparewait")
            self.spare_owner[nop.ins.name] = key
            if self.last.get(engine) is not None:
                self.tile.add_dep_helper(nop.ins, self.last[engine], False, "pin")
            self.last[engine] = nop.ins
            nops.append(nop)
        inst = getattr(eng, method)(*args, **kw)
        # pin the real instruction after its spares so the scheduler cannot
        # float a spare (and the wait it carries) past its owner
        for nop in nops:
            self.tile.add_dep_helper(inst.ins, nop.ins, False, "pin")
        self.inst_key[inst.ins.name] = key
        self.last[engine] = inst.ins
        return inst

    def plant_tail(self):
        for (kind, engine), n in self.needed.items():
            if kind != "tail":
                continue
            eng = getattr(self.nc, engine)
            for _ in range(n):
                nop = eng.nop(nofuse=True, hint="tailspare")
                self.spare_owner[nop.ins.name] = ("tail", engine)
                if self.last.get(engine) is not None:
                    self.tile.add_dep_helper(nop.ins, self.last[engine], False, "pin")
                self.last[engine] = nop.ins


def redistribute_waits(nc, em, mybir, max_waits=1):
    """Move excess sync waits onto the spare NoOps planted for each
    instruction (pinned directly before it on the same engine queue).
    Returns dict key -> spare count still needed."""
    by_owner = {}
    deficit = {}
    for blk in nc.m.functions[0].blocks:
        for ins in blk.instructions:
            own = em.spare_owner.get(ins.name)
            if own is not None:
                by_owner.setdefault(own, []).append(ins)
    for blk in nc.m.functions[0].blocks:
        for ins in blk.instructions:
            if ins.name in em.spare_owner:
                continue
            nm = type(ins).__name__
            if nm in _EXEMPT:
                continue
            if nm == "InstDrain":
                key = ("tail", _ENG_ATTR.get(str(ins.engine), "sync"))
            else:
                key = em.inst_key.get(ins.name)
                if key is None:
                    continue
            si = ins.sync_info
            waits = list(si.on_wait) if si is not None else []
            if len(waits) <= max_waits:
                continue
            excess = waits[: len(waits) - max_waits]
            keep = waits[len(waits) - max_waits :]
            mine = [
                s
                for s in by_owner.get(key, [])
                if not (s.sync_info and s.sync_info.on_wait)
            ]
            if len(excess) > len(mine):
                deficit[key] = deficit.get(key, 0) + len(waits) - max_waits
                continue
            for w, sp in zip(excess, mine):
                old = sp.sync_info
                ou = list(old.on_update) if old is not None else []
                sp.sync_info = mybir.SyncInfo(on_wait=[w], on_update=ou)
            ins.sync_info = mybir.SyncInfo(
                on_wait=keep, on_update=list(si.on_update) if si else []
            )
    return deficit


def check_wait_budget(nc, em, mybir, max_waits=1):
    bad = []
    for blk in nc.m.functions[0].blocks:
        for ins in blk.instructions:
            if type(ins).__name__ in _EXEMPT:
                continue
            si = ins.sync_info
            if si is not None and len(si.on_wait) > max_waits:
                bad.append((ins.name, type(ins).__name__, len(si.on_wait)))
    return bad


def _build_nc(needed, has_bq=False):
    import concourse.bass as bass
    import concourse.tile as tile
    from concourse import mybir

    f32 = mybir.dt.float32
    f16 = mybir.dt.float16
    f8 = mybir.dt.float8e4
    DR = mybir.MatmulPerfMode.DoubleRow

    nc = bass.Bass()

    x_d = nc.dram_tensor("x", [C, H], f16, kind="ExternalInput")
    # wall[slab] = [mt pair 0 | mt pair 1 | wpv pair 0 | wpv pair 1] fp8
    # DoubleRow slabs: slab[:, i, :] is the (2t+i)-th 128-row chunk of
    # 16*MT (MT = wq.T @ wk) resp. 16*WPV^T (WPV = wp @ wv).
    wall_d = nc.dram_tensor("wall", [P, 4, 2, FD], f8, kind="ExternalInput")
    # consts = [gamma | beta | bp2 | ind_g/16] columns
    consts_d = nc.dram_tensor("consts", [P, 3 * NCH + GPC], f32, kind="ExternalInput")
    indb_d = nc.dram_tensor("ind_b", [GPC, P], f32, kind="ExternalInput")
    u_d = nc.dram_tensor("u8", [P, NCH], f8, kind="ExternalInput")
    out_d = nc.dram_tensor("out", [C, H], f32, kind="ExternalOutput")

    from contextlib import ExitStack

    with tile.TileContext(nc) as tc, ExitStack() as ctx:
        em = _Emitter(nc, tile, needed)
        res = ctx.enter_context(tc.tile_pool(name="res", bufs=1))
        work = ctx.enter_context(tc.tile_pool(name="work", bufs=2))
        ps_st = ctx.enter_context(tc.tile_pool(name="ps_st", bufs=4, space="PSUM"))
        ps_mm = ctx.enter_context(tc.tile_pool(name="ps_mm", bufs=3, space="PSUM"))
        ps_aux = ctx.enter_context(tc.tile_pool(name="ps_aux", bufs=1, space="PSUM"))

        # ---- resident SBUF tiles ----
        xh = [res.tile([P, H], f16, tag=f"xh{t}", name=f"xh{t}") for t in range(NCH)]
        xq8 = [
            res.tile([P, 2, H], f8, tag=f"xq8{t}", name=f"xq8{t}") for t in range(NPR)
        ]
        zt8 = [
            res.tile([P, 2, H], f8, tag=f"zt8{t}", name=f"zt8{t}") for t in range(NPR)
        ]
        vT8 = [
            res.tile([P, 2, FD], f8, tag=f"vT8{t}", name=f"vT8{t}") for t in range(NJP)
        ]
        mt8 = [
            res.tile([P, 2, FD], f8, tag=f"mt8{t}", name=f"mt8{t}") for t in range(NPR)
        ]
        wpv8 = [
            res.tile([P, 2, FD], f8, tag=f"wpv8{t}", name=f"wpv8{t}")
            for t in range(NPR)
        ]
        consts_s = res.tile([P, 3 * NCH + GPC], f32, tag="consts")
        gamma_s = consts_s[:, 0 * NCH : 1 * NCH]
        beta_s = consts_s[:, 1 * NCH : 2 * NCH]
        bp2_s = consts_s[:, 2 * NCH : 3 * NCH]
        indg_s = consts_s[:, 3 * NCH : 3 * NCH + GPC]
        indb_s = res.tile([GPC, P], f32, tag="indb")
        u_s = res.tile([P, NCH], f8, tag="u8") if has_bq else None
        g_s = res.tile([P, NJT], f32, tag="g_s") if has_bq else None
        ones128 = res.tile([P, 2, P], f8, tag="ones128")
        stat = res.tile([P, 2 * NCH], f32, tag="stat")
        stats2 = res.tile([GPC, 2 * NCH], f32, tag="stats2")
        a_s = res.tile([P, NCH], f32, tag="a_s")
        b_s = res.tile([P, NCH], f32, tag="b_s")
        eps_s = res.tile([GPC, 1], f32, tag="eps")
        ebias_s = res.tile([P, 1], f32, tag="ebias")

        AF = mybir.ActivationFunctionType
        OP = mybir.AluOpType

        # ---- phase A: loads + groupnorm stats + fp8 quantize ----
        # Big-chunk x DMAs (small DMAs serialize on the ~0.6us per-issue
        # queue cost). The sync queue delivers ~270GB/s, the gpsimd queue
        # only ~130GB/s. The stats chains gate everything, so the two
        # chunks that anchor them (t0 for the DVE bn chain, t1 for the
        # serial ACT Copy/Square accum) are split across BOTH queues to
        # land earliest; t2/t3 ride sync whole. gpsimd then carries
        # consts + the fp8 weight slabs (mt pairs first -- the z GEMM
        # needs them; total ~1.0MB, under the DVFS-demotion hazard).
        # all engines idle in the head, so the three DMA-capable queues
        # (sync/SP, scalar/ACT, gpsimd) run in parallel: sync+scalar
        # split the two stats-anchoring chunks (t0 for DVE bn, t1 for
        # the serial ACT accum), sync then carries t2/t3, gpsimd alone
        # carries consts + weights (arrives ~5us, far ahead of the z
        # GEMM).
        em("sync", "dma_start", out=xh[0][:, 0 : 2 * FD], in_=x_d[0:P, 0 : 2 * FD])
        em("scalar", "dma_start", out=xh[0][:, 2 * FD :], in_=x_d[0:P, 2 * FD :])
        em("sync", "dma_start", out=xh[1][:, 0 : 2 * FD], in_=x_d[P : 2 * P, 0 : 2 * FD])
        em("scalar", "dma_start", out=xh[1][:, 2 * FD :], in_=x_d[P : 2 * P, 2 * FD :])
        em("sync", "dma_start", out=xh[2], in_=x_d[2 * P : 3 * P, :])
        em("sync", "dma_start", out=xh[3], in_=x_d[3 * P : 4 * P, :])
        em("gpsimd", "dma_start", out=consts_s, in_=consts_d[:, :])
        em("gpsimd", "dma_start", out=indb_s, in_=indb_d[:, :])
        if has_bq:
            em("gpsimd", "dma_start", out=u_s, in_=u_d[:, :])
        for t in range(NPR):
            em("gpsimd", "dma_start", out=mt8[t], in_=wall_d[:, t, :, :])
        for t in range(NPR):
            em("gpsimd", "dma_start", out=wpv8[t], in_=wall_d[:, 2 + t, :, :])
        em("vector", "memset", eps_s, EPS)
        em("vector", "memset", ebias_s, EXP_BIAS)
        em("gpsimd", "memset", ones128, SC_W)

        # per-channel mean / E[x^2]: DVE bn_stats for t0,t2,t3 in DMA
        # arrival order; chunk t1 on ACT via accum_out (Copy -> sum(x),
        # Square -> sum(x^2)), dumping the main output into a scratch
        # tile. (Giving ACT a second chunk measured ~1.5us SLOWER: its
        # four serial 2us accum passes overtake the DVE chain.)
        sx3 = work.tile([P, 4], f32, tag="sx3", name="sx3")
        xdump = work.tile([P, H], f16, tag="xdump", name="xdump")
        for k, t in enumerate((1,)):
            em(
                "scalar",
                "activation",
                out=xdump,
                in_=xh[t],
                func=AF.Copy,
                accum_out=sx3[:, 2 * k : 2 * k + 1],
            )
            em(
                "scalar",
                "activation",
                out=xdump,
                in_=xh[t],
                func=AF.Square,
                accum_out=sx3[:, 2 * k + 1 : 2 * k + 2],
            )
        for t in (0, 2, 3):
            bn6 = work.tile([P, 4, 6], f32, tag="bn6", name="bn6")
            for sg in range(4):
                em(
                    "vector",
                    "bn_stats",
                    out=bn6[:, sg, :],
                    in_=xh[t][:, sg * FD : (sg + 1) * FD],
                )
            mv = work.tile([P, 2], f32, tag="mv", name="mv")
            em("vector", "bn_aggr", out=mv, in_=bn6)
            em("vector", "tensor_copy", out=stat[:, t : t + 1], in_=mv[:, 0:1])
            # E[x^2] = var + mean^2 in one fused op
            em(
                "vector",
                "scalar_tensor_tensor",
                out=stat[:, NCH + t : NCH + t + 1],
                in0=mv[:, 0:1],
                scalar=mv[:, 0:1],
                in1=mv[:, 1:2],
                op0=OP.mult,
                op1=OP.add,
            )
        for k, t in enumerate((1,)):
            em(
                "vector",
                "tensor_scalar_mul",
                stat[:, t : t + 1],
                sx3[:, 2 * k : 2 * k + 1],
                1.0 / H,
            )
            em(
                "vector",
                "tensor_scalar_mul",
                stat[:, NCH + t : NCH + t + 1],
                sx3[:, 2 * k + 1 : 2 * k + 2],
                1.0 / H,
            )
        # indg holds 1/16 so this directly yields group means of [m, E[x^2]]
        g_ps = ps_aux.tile([GPC, 2 * NCH], f32, tag="aux", name="gps")
        em("tensor", "matmul", g_ps, lhsT=indg_s, rhs=stat, start=True, stop=True)
        em("vector", "tensor_copy", out=stats2, in_=g_ps)
        m2g = work.tile([GPC, NCH], f32, tag="m2g", name="m2g")
        em("vector", "tensor_mul", out=m2g, in0=stats2[:, 0:NCH], in1=stats2[:, 0:NCH])
        # 1/sqrt(v+eps) as sqrt(1/(v+eps)): eps folds into the variance
        # subtraction and both DVE ops run back-to-back (one less
        # cross-engine hop on the serial a/b chain)
        var_t = work.tile([GPC, NCH], f32, tag="var", name="var")
        em(
            "vector",
            "scalar_tensor_tensor",
            out=var_t,
            in0=stats2[:, NCH :],
            scalar=float(EPS),
            in1=m2g,
            op0=OP.add,
            op1=OP.subtract,
        )
        srt = work.tile([GPC, NCH], f32, tag="srt", name="srt")
        em("vector", "reciprocal", out=srt, in_=var_t)
        em("scalar", "activation", out=stats2[:, NCH :], in_=srt, func=AF.Sqrt)
        bc_ps = ps_aux.tile([P, 2 * NCH], f32, tag="aux", name="bcps")
        em("tensor", "matmul", bc_ps, lhsT=indb_s, rhs=stats2, start=True, stop=True)
        em("vector", "tensor_mul", out=a_s, in0=bc_ps[:, NCH : 2 * NCH], in1=gamma_s)
        tmp_ma = work.tile([P, NCH], f32, tag="tmp_ma", name="tmp_ma")
        em("vector", "tensor_mul", out=tmp_ma, in0=bc_ps[:, 0:NCH], in1=a_s)
        em("vector", "tensor_sub", out=b_s, in0=beta_s, in1=tmp_ma)
        # xq8 = fp8(a*x + b) into DoubleRow pair tiles, sg-major so the z
        # GEMM can chase slice by slice; alternate DVE/ACT. Raw x stays in
        # xh for the residual.
        for sg in range(4):
            for t in range(NCH):
                sl = slice(sg * FD, (sg + 1) * FD)
                osl = xq8[t // 2][:, t % 2, sl]
                if t % 2 == 0:
                    em(
                        "vector",
                        "tensor_scalar",
                        out=osl,
                        in0=xh[t][:, sl],
                        scalar1=a_s[:, t : t + 1],
                        scalar2=b_s[:, t : t + 1],
                        op0=OP.mult,
                        op1=OP.add,
                    )
                else:
                    em(
                        "scalar",
                        "activation",
                        out=osl,
                        in_=xh[t][:, sl],
                        func=AF.Identity,
                        scale=a_s[:, t : t + 1],
                        bias=b_s[:, t : t + 1],
                    )

        # ---- phase B: z8 = fp8(16 M @ hn) (n-major to chase the quantize) ----
        ev = 0
        for n in range(NIT):
            for a in range(NCH):
                ps = ps_mm.tile([P, FD], f32, tag="mm", name="mmps")
                for t in range(NPR):
                    em(
                        "tensor",
                        "matmul",
                        ps,
                        lhsT=mt8[t][:, :, a * P : (a + 1) * P],
                        rhs=xq8[t][:, :, n * FD : (n + 1) * FD],
                        start=(t == 0),
                        stop=(t == NPR - 1),
                        perf_mode=DR,
                    )
                osl = zt8[a // 2][:, a % 2, n * FD : (n + 1) * FD]
                # all PSUM->fp8 quantize copies ride DVE: ACT is the
                # co-critical engine (64 exps saturate it through phase D)
                em("vector", "tensor_copy", out=osl, in_=ps)
                ev += 1

        # ---- phase C: vT8 = fp8(hn^T (16 WPV^T)) GEMM ----
        for j in range(NJT):
            ps = ps_mm.tile([P, FD], f32, tag="mm", name="mmps")
            for t in range(NPR):
                em(
                    "tensor",
                    "matmul",
                    ps,
                    lhsT=xq8[t][:, :, j * P : (j + 1) * P],
                    rhs=wpv8[t],
                    start=(t == 0),
                    stop=(t == NPR - 1),
                    perf_mode=DR,
                )
            osl = vT8[j // 2][:, j % 2, :]
            em("vector", "tensor_copy", out=osl, in_=ps)
            ev += 1
        if has_bq:
            # g[j] = (Wk.T bq) . hn[:, j] * SCL - 2 ln 2, added to scores
            # inside the exp (u8 carries 16 Wk.T bq in fp8).
            for j in range(NJT):
                gp = ps_aux.tile([P, 1], f32, tag="aux", name="gps1")
                for c in range(NCH):
                    em(
                        "tensor",
                        "matmul",
                        gp,
                        lhsT=xq8[c // 2][:, c % 2, j * P : (j + 1) * P],
                        rhs=u_s[:, c : c + 1],
                        start=(c == 0),
                        stop=(c == NCH - 1),
                    )
                em(
                    "vector",
                    "tensor_scalar",
                    out=g_s[:, j : j + 1],
                    in0=gp,
                    scalar1=SCL / SC_W,
                    scalar2=EXP_BIAS,
                    op0=OP.mult,
                    op1=OP.add,
                )

        # ---- phase D: attention + proj per i-tile ----
        for it in range(NIT):
            i0 = it * FD
            est8 = [
                work.tile([P, 2, FD], f8, tag=f"est{t}", name=f"est{t}")
                for t in range(NJP)
            ]
            rb_s = work.tile([P, FD], f32, tag="rb_s", name="rb_s")
            rb_ps = ps_aux.tile([P, FD], f32, tag="aux", name="rbps")
            for j in range(NJT):
                ps = ps_st.tile([P, FD], f32, tag="st", name="stps")
                for t in range(NPR):
                    em(
                        "tensor",
                        "matmul",
                        ps,
                        lhsT=xq8[t][:, :, j * P : (j + 1) * P],
                        rhs=zt8[t][:, :, i0 : i0 + FD],
                        start=(t == 0),
                        stop=(t == NPR - 1),
                        perf_mode=DR,
                    )
                em(
                    "scalar",
                    "activation",
                    out=est8[j // 2][:, j % 2, :],
                    in_=ps,
                    func=AF.Exp,
                    scale=SCL / SC_W,
                    bias=g_s[:, j : j + 1] if has_bq else ebias_s,
                )
            # denominator: the PE itself sums est8 pairs over partitions
            # via DoubleRow ones-matmuls (ones128 holds 16.0, folding
            # the vp scale) accumulated into rb_ps -- a DVE/Pool add
            # tree over the fp8 tiles is 1.4-1.9us per [128,1024] add
            # and stalled the PE 7-8us per i-tile at the den handoff.
            # Only the last ones-matmul can wait on ACT (~0.7us for the
            # final exp); the rest consume long-finished est8 pairs.
            for t in range(NJP):
                em(
                    "tensor",
                    "matmul",
                    rb_ps,
                    lhsT=ones128,
                    rhs=est8[t],
                    start=(t == 0),
                    stop=(t == NJP - 1),
                    perf_mode=DR,
                )
            # 1/(16 den) on ACT as exp(-ln(x)) (bass blocks
            # AF.Reciprocal; the ~1e-3 LUT error is far under the fp8
            # noise). DVE's reciprocal takes ~4us/[128,512] and sat on
            # the PSUM-bank recycle path, stalling the PE ~5us per
            # i-tile.
            rb_ln = work.tile([P, FD], f32, tag="rb_ln", name="rb_ln")
            em("scalar", "activation", out=rb_ln, in_=rb_ps, func=AF.Ln)
            em(
                "scalar",
                "activation",
                out=rb_s,
                in_=rb_ln,
                func=AF.Exp,
                scale=-1.0,
            )
            # out[o, i] = (sum_j 16 vp[j, o] est[j, i]) / (16 den) + bp2 + x
            # o_s chains alternate DVE/Pool so the last i-tile's serial
            # epilogue halves.
            for o in range(NCH):
                ps = ps_mm.tile([P, FD], f32, tag="mm", name="mmps")
                for t in range(NJP):
                    em(
                        "tensor",
                        "matmul",
                        ps,
                        lhsT=vT8[t][:, :, o * P : (o + 1) * P],
                        rhs=est8[t],
                        start=(t == 0),
                        stop=(t == NJP - 1),
                        perf_mode=DR,
                    )
                # both ops stay on DVE: GPSIMD cannot read PSUM (mul) and
                # TensorScalarPtr is not a Pool opcode (stt).
                o_s = work.tile([P, FD], f32, tag=f"o_s{o}", name=f"o_s{o}")
                em("vector", "tensor_mul", out=o_s, in0=ps, in1=rb_s)
                em(
                    "vector",
                    "scalar_tensor_tensor",
                    out=o_s,
                    in0=o_s,
                    scalar=bp2_s[:, o : o + 1],
                    in1=xh[o][:, i0 : i0 + FD],
                    op0=OP.add,
                    op1=OP.add,
                )
                # the last i-tile's outputs split across the scalar and
                # sync queues (both idle by then, fast drain); gpsimd's
                # drain costs ~3.7us so it only carries mid-kernel tiles.
                # (Routing ALL mid-tile outputs over sync alone measured
                # ~1us slower -- keep the two-queue split.)
                if it == NIT - 1:
                    q = "sync" if o % 2 == 0 else "scalar"
                else:
                    q = "sync" if o % 2 == 0 else "gpsimd"
                em(
                    q,
                    "dma_start",
                    out=out_d[o * P : (o + 1) * P, i0 : i0 + FD],
                    in_=o_s,
                )
        em.plant_tail()

    from concourse import mybir as _mybir

    deficit = redistribute_waits(nc, em, _mybir)
    return nc, em, deficit


_BUILT_MAP = {}


def get_built(has_bq=False):
    if has_bq not in _BUILT_MAP:
        needed = {}
        deficit = None
        for attempt in range(8):
            nc, em, deficit = _build_nc(dict(needed), has_bq=has_bq)
            if not deficit:
                break
            for key, n in deficit.items():
                needed[key] = max(needed.get(key, 0), n)
        else:
            raise RuntimeError(f"spare-wait fixpoint did not converge: {deficit}")
        from concourse import mybir

        bad = check_wait_budget(nc, em, mybir)
        if bad:
            raise RuntimeError(f"instructions over wait budget: {bad[:10]}")
        _BUILT_MAP[has_bq] = nc
    return _BUILT_MAP[has_bq]


def _host_prep(x, gamma, beta, wq, bq, wk, bk, wv, bv, wp, bp):
    import ml_dtypes

    f = np.float32
    f8 = ml_dtypes.float8_e4m3

    def t128(v):  # [512] -> [128, 4] with element (p, t) = v[t*128 + p]
        return np.ascontiguousarray(v.reshape(NCH, P).T.astype(f))

    ind_g = np.zeros((P, GPC), f)
    ind_g[np.arange(P), np.arange(P) // CPG] = 1.0 / CPG
    ind_b = np.zeros((GPC, P), f)
    ind_b[np.arange(P) // CPG, np.arange(P)] = 1.0
    bp2 = wp.astype(f) @ bv.astype(f) + bp.astype(f)
    # M = Wk.T @ Wq folds the q and k GEMMs into one (scores are bilinear in
    # hn); lhsT layout needs MT = M.T. bk cancels in the softmax; bq needs
    # the u-correction path (zero in this problem). 16x host scale keeps
    # the ~N(0,1/512) entries clear of the e4m3 subnormal cutoff.
    mt = (SC_W * (wq.astype(np.float64).T @ wk.astype(np.float64))).astype(f8)
    u = (wk.astype(np.float64).T @ bq.astype(np.float64)).astype(f)
    wpvT = (SC_W * (wp.astype(np.float64) @ wv.astype(np.float64)).T).astype(f8)

    def pair(w, t):  # [128, 2, 512] DoubleRow slab for chunks (2t, 2t+1)
        return np.stack(
            [w[(2 * t) * P : (2 * t + 1) * P, :], w[(2 * t + 1) * P : (2 * t + 2) * P, :]],
            axis=1,
        )

    wall = np.stack(
        [pair(mt, 0), pair(mt, 1), pair(wpvT, 0), pair(wpvT, 1)], axis=1
    )  # [P, 4, 2, FD] fp8
    consts = np.concatenate([t128(gamma), t128(beta), t128(bp2), ind_g], axis=1)
    shared = {
        "wall": np.ascontiguousarray(wall),
        "consts": np.ascontiguousarray(consts),
        "ind_b": ind_b,
        "u8": np.ascontiguousarray((SC_W * t128(u)).astype(f8)),
    }
    return [
        {"x": np.ascontiguousarray(x[b].astype(np.float16)), **shared}
        for b in range(B)
    ], bool(np.any(bq != 0))


def run(inputs, trace=False, **kw):
    from concourse.bass_utils import run_bass_kernel_spmd

    in_maps, has_bq = _host_prep(**{k: np.asarray(v) for k, v in inputs.items()})
    nc = get_built(has_bq=has_bq)
    res = run_bass_kernel_spmd(nc, in_maps, list(range(B)), trace=trace, **kw)
    out = np.stack([res.results[b]["out"] for b in range(B)]).astype(np.float32)
    return out, res


def kernel(**inputs):
    out, _ = run(inputs, trace=False)
    return out


# revision 31
# speedup vs baseline: 1.1718x; 1.1718x over previous
"""AttnBlock (GroupNorm -> QKV 1x1 -> attention over H -> proj -> residual)
for B=8, C=512, H=2048 on 8 trn2 NeuronCores, data-parallel over batch.

Each core runs the full block for one batch element. All four heavy GEMMs
run as fp8e4 (TRN e4m3, max 240) DoubleRow matmuls: two 128-deep K-chunks
packed per instruction (lhsT [K,2,M], rhs [K,2,N]) at 2x the fp16 row
rate, with f32 PSUM accumulation. The folded weights MT = Wk^T Wq and
WPV^T = (Wp Wv)^T are scaled by 16 on the host so their ~N(0, 1/512)
entries clear the e4m3 subnormal cutoff (2^-6) with a >2x headroom under
the 240 max on the GEMM outputs (|16 z|, |16 vp| <~ 130). The 16x scales
fold into the exp scale (SCL/16) and the denominator ones-matmul constant
(16.0). exp gets a -2 ln 2 bias so fp8 est tops out ~75 « 240; the shift
cancels exactly in the softmax ratio. CPU sim: ~1.0e-2 scale-relative
error vs the f32 reference (gate 2e-2; fp16 baseline was 3.8e-4).

Structure per core:
  A. x as fp16 over three parallel DMA queues (sync+scalar split the
     two stats-anchoring chunks; gpsimd alone carries consts + the fp8
     weight slabs, mt pairs first -- the z GEMM needs them earliest);
     group-norm stats on DVE bn_stats for 3 chunks + ACT accum_out
     (Copy/Square) for one; xq8 = fp8(a*x + b) quantize pass, sg-major,
     into DoubleRow pair tiles [P,2,H] (x stays resident in fp16 for
     the residual).
  B. z8 = fp8(16 * M @ hn) via 2 DoubleRow matmuls per [128,512] tile
     (M = Wk^T Wq folds the q and k GEMMs; scores are bilinear in hn).
  C. vT8 = fp8((hn^T (16 WPV^T))) with WPV = Wp @ Wv: since sum_j
     softmax = 1, the proj GEMM folds away entirely -- out =
     (WPV hn) @ probs^T / den + bp2 + x with bp2 = Wp bv + bp.
  D. per 512-wide i-tile: scores^T via 2 DoubleRow matmuls per j-chunk
     -> exp -> est8 (fp8 pair tiles, write-only for ACT); denominator
     = 8 more DoubleRow ones-matmuls on the PE itself (sums over
     partitions AND broadcasts to all 128; any DVE/Pool add tree over
     fp8 tiles measured 1.4-1.9us per add and stalled the PE 7-8us per
     i-tile); 1/(16 den) as exp(-ln) on ACT (DVE reciprocal is ~4us
     per [128,512] and sat on the PSUM-bank recycle path); output
     chunks = 8 DoubleRow matmuls, scaled by 1/(16 den) on DVE + bias
     + residual; final i-tile outputs split across the sync and scalar
     queues (both idle by then, fast drain).

Engine balance at ~116us wall (vs 174us fp16 baseline): PE ~80us busy
(352 matmuls, 216ns each at full clock, LDWEIGHTS fully hidden), ACT
~72us (64 exps at ~730ns dominate), DVE ~67us, head ~22us (startup +
DMA + serial stats chain), tail ~8us.

Known hazards on this stack (empirical): long PE warmup streams or a
heavily loaded gpsimd DMA queue trigger a chip-wide ~1.2x DVFS clock
demotion for the rest of the kernel; inserting extra PE/ACT ops inside
the stats chain (dummy table-load or warmup matmuls) hangs execution.

Walrus limits compute-engine instructions to one sync wait each, while Tile
emits all required waits on the first instruction that needs them. We route
every emitted op through a wrapper that can plant same-engine spare NoOps
directly before it; a post-scheduling pass moves excess waits onto the
spares, and a build fixpoint discovers where spares are needed.
"""

import sys

sys.path.insert(0, "/opt/trn_rl_repo")

import numpy as np

B, C, H = 8, 512, 2048
GROUPS = 32
EPS = 1e-6
P = 128
FD = 512
NCH = C // P  # 4 channel chunks
NPR = NCH // 2  # 2 DoubleRow channel-pair tiles
NJT = H // P  # 16 j-chunks
NJP = NJT // 2  # 8 DoubleRow j-pair tiles
NIT = H // FD  # 4 i-tiles
GPC = 8  # groups per 128-channel chunk (128/16)
CPG = C // GROUPS  # 16 channels per group
SCL = float(C) ** -0.5
SC_W = 16.0  # host-side scale on MT and WPV^T for fp8 range
EXP_BIAS = float(-2.0 * np.log(2.0))  # cancels in softmax; keeps est < 240

# instruction types never subject to the 1-wait redistribution
_EXEMPT = ("InstEventSemaphore", "InstNoOp")

_ENG_ATTR = {
    "EngineType.PE": "tensor",
    "EngineType.DVE": "vector",
    "EngineType.Activation": "scalar",
    "EngineType.Pool": "gpsimd",
    "EngineType.SP": "sync",
}


class _Emitter:
    """Emission wrapper: plants pinned spare NoOps before instructions that
    the previous build iteration found to carry >1 sync wait."""

    def __init__(self, nc, tile_mod, needed):
        self.nc = nc
        self.tile = tile_mod
        self.needed = needed
        self.spare_owner = {}  # spare inst name -> key
        self.inst_key = {}  # real inst name -> key
        self.counters = {}
        self.last = {}

    def __call__(self, engine, method, *args, **kw):
        eng = getattr(self.nc, engine)
        idx = self.counters.get(engine, 0)
        self.counters[engine] = idx + 1
        key = (engine, idx)
        nops = []
        for _ in range(self.needed.get(key, 0) if key[0] != "tail" else 0):
            nop = eng.nop(nofuse=True, hint="sparewait")
            self.spare_owner[nop.ins.name] = key
            if self.last.get(engine) is not None:
                self.tile.add_dep_helper(nop.ins, self.last[engine], False, "pin")
            self.last[engine] = nop.ins
            nops.append(nop)
        inst = getattr(eng, method)(*args, **kw)
        # pin the real instruction after its spares so the scheduler cannot
        # float a spare (and the wait it carries) past its owner
        for nop in nops:
            self.tile.add_dep_helper(inst.ins, nop.ins, False, "pin")
        self.inst_key[inst.ins.name] = key
        self.last[engine] = inst.ins
        return inst

    def plant_tail(self):
        for (kind, engine), n in self.needed.items():
            if kind != "tail":
                continue
            eng = getattr(self.nc, engine)
            for _ in range(n):
                nop = eng.nop(nofuse=True, hint="tailspare")
                self.spare_owner[nop.ins.name] = ("tail", engine)
                if self.last.get(engine) is not None:
                    self.tile.add_dep_helper(nop.ins, self.last[engine], False, "pin")
                self.last[engine] = nop.ins


def redistribute_waits(nc, em, mybir, max_waits=1):
    """Move excess sync waits onto the spare NoOps planted for each
    instruction (pinned directly before it on the same engine queue).
    Returns dict key -> spare count still needed."""
    by_owner = {}
    deficit = {}
    for blk in nc.m.functions[0].blocks:
        for ins in blk.instructions:
            own = em.spare_owner.get(ins.name)
            if own is not None:
                by_owner.setdefault(own, []).append(ins)
    for blk in nc.m.functions[0].blocks:
        for ins in blk.instructions:
            if ins.name in em.spare_owner:
                continue
            nm = type(ins).__name__
            if nm in _EXEMPT:
                continue
            if nm == "InstDrain":
                key = ("tail", _ENG_ATTR.get(str(ins.engine), "sync"))
            else:
                key = em.inst_key.get(ins.name)
                if key is None:
                    continue
            si = ins.sync_info
            waits = list(si.on_wait) if si is not None else []
            if len(waits) <= max_waits:
                continue
            excess = waits[: len(waits) - max_waits]
            keep = waits[len(waits) - max_waits :]
            mine = [
                s
                for s in by_owner.get(key, [])
                if not (s.sync_info and s.sync_info.on_wait)
            ]
            if len(excess) > len(mine):
                deficit[key] = deficit.get(key, 0) + len(waits) - max_waits
                continue
            for w, sp in zip(excess, mine):
                old = sp.sync_info
                ou = list(old.on_update) if old is not None else []
                sp.sync_info = mybir.SyncInfo(on_wait=[w], on_update=ou)
            ins.sync_info = mybir.SyncInfo(
                on_wait=keep, on_update=list(si.on_update) if si else []
            )
    return deficit


def check_wait_budget(nc, em, mybir, max_waits=1):
    bad = []
    for blk in nc.m.functions[0].blocks:
        for ins in blk.instructions:
            if type(ins).__name__ in _EXEMPT:
                continue
            si = ins.sync_info
            if si is not None and len(si.on_wait) > max_waits:
                bad.append((ins.name, type(ins).__name__, len(si.on_wait)))
    return bad


def _build_nc(needed, has_bq=False):
    import concourse.bass as bass
    import concourse.tile as tile
    from concourse import mybir

    f32 = mybir.dt.float32
    f16 = mybir.dt.float16
    f8 = mybir.dt.float8e4
    DR = mybir.MatmulPerfMode.DoubleRow

    nc = bass.Bass()

    x_d = nc.dram_tensor("x", [C, H], f16, kind="ExternalInput")
    # wall[slab] = [mt pair 0 | mt pair 1 | wpv pair 0 | wpv pair 1] fp8
    # DoubleRow slabs: slab[:, i, :] is the (2t+i)-th 128-row chunk of
    # 16*MT (MT = wq.T @ wk) resp. 16*WPV^T (WPV = wp @ wv).
    wall_d = nc.dram_tensor("wall", [P, 4, 2, FD], f8, kind="ExternalInput")
    # consts = [gamma | beta | bp2 | ind_g/16] columns
    consts_d = nc.dram_tensor("consts", [P, 3 * NCH + GPC], f32, kind="ExternalInput")
    indb_d = nc.dram_tensor("ind_b", [GPC, P], f32, kind="ExternalInput")
    u_d = nc.dram_tensor("u8", [P, NCH], f8, kind="ExternalInput")
    out_d = nc.dram_tensor("out", [C, H], f32, kind="ExternalOutput")

    from contextlib import ExitStack

    with tile.TileContext(nc) as tc, ExitStack() as ctx:
        em = _Emitter(nc, tile, needed)
        res = ctx.enter_context(tc.tile_pool(name="res", bufs=1))
        work = ctx.enter_context(tc.tile_pool(name="work", bufs=2))
        ps_st = ctx.enter_context(tc.tile_pool(name="ps_st", bufs=4, space="PSUM"))
        ps_mm = ctx.enter_context(tc.tile_pool(name="ps_mm", bufs=3, space="PSUM"))
        ps_aux = ctx.enter_context(tc.tile_pool(name="ps_aux", bufs=1, space="PSUM"))

        # ---- resident SBUF tiles ----
        xh = [res.tile([P, H], f16, tag=f"xh{t}", name=f"xh{t}") for t in range(NCH)]
        xq8 = [
            res.tile([P, 2, H], f8, tag=f"xq8{t}", name=f"xq8{t}") for t in range(NPR)
        ]
        zt8 = [
            res.tile([P, 2, H], f8, tag=f"zt8{t}", name=f"zt8{t}") for t in range(NPR)
        ]
        vT8 = [
            res.tile([P, 2, FD], f8, tag=f"vT8{t}", name=f"vT8{t}") for t in range(NJP)
        ]
        mt8 = [
            res.tile([P, 2, FD], f8, tag=f"mt8{t}", name=f"mt8{t}") for t in range(NPR)
        ]
        wpv8 = [
            res.tile([P, 2, FD], f8, tag=f"wpv8{t}", name=f"wpv8{t}")
            for t in range(NPR)
        ]
        consts_s = res.tile([P, 3 * NCH + GPC], f32, tag="consts")
        gamma_s = consts_s[:, 0 * NCH : 1 * NCH]
        beta_s = consts_s[:, 1 * NCH : 2 * NCH]
        bp2_s = consts_s[:, 2 * NCH : 3 * NCH]
        indg_s = consts_s[:, 3 * NCH : 3 * NCH + GPC]
        indb_s = res.tile([GPC, P], f32, tag="indb")
        u_s = res.tile([P, NCH], f8, tag="u8") if has_bq else None
        g_s = res.tile([P, NJT], f32, tag="g_s") if has_bq else None
        ones128 = res.tile([P, 2, P], f8, tag="ones128")
        stat = res.tile([P, 2 * NCH], f32, tag="stat")
        stats2 = res.tile([GPC, 2 * NCH], f32, tag="stats2")
        a_s = res.tile([P, NCH], f32, tag="a_s")
        b_s = res.tile([P, NCH], f32, tag="b_s")
        eps_s = res.tile([GPC, 1], f32, tag="eps")
        ebias_s = res.tile([P, 1], f32, tag="ebias")

        AF = mybir.ActivationFunctionType
        OP = mybir.AluOpType

        # ---- phase A: loads + groupnorm stats + fp8 quantize ----
        # Big-chunk x DMAs (small DMAs serialize on the ~0.6us per-issue
        # queue cost). The sync queue delivers ~270GB/s, the gpsimd queue
        # only ~130GB/s. The stats chains gate everything, so the two
        # chunks that anchor them (t0 for the DVE bn chain, t1 for the
        # serial ACT Copy/Square accum) are split across BOTH queues to
        # land earliest; t2/t3 ride sync whole. gpsimd then carries
        # consts + the fp8 weight slabs (mt pairs first -- the z GEMM
        # needs them; total ~1.0MB, under the DVFS-demotion hazard).
        # the gpsimd (SWDGE) queue is COMPLETELY unused for DMA: its
        # end-of-kernel drain costs ~3.5us (10x the other engines') once
        # the queue sees any traffic, and that drain lands inside the
        # measured exec window. The two HWDGE queues (sync/SP,
        # scalar/ACT) carry everything: they split the two
        # stats-anchoring chunks (t0 for DVE bn, t1 for the serial ACT
        # accum), then sync takes t2/t3 and scalar takes consts + the
        # fp8 weight slabs (mt pairs first -- the z GEMM needs them
        # earliest, ~25us in).
        em("sync", "dma_start", out=xh[0][:, 0 : 2 * FD], in_=x_d[0:P, 0 : 2 * FD])
        em("scalar", "dma_start", out=xh[0][:, 2 * FD :], in_=x_d[0:P, 2 * FD :])
        em("sync", "dma_start", out=xh[1][:, 0 : 2 * FD], in_=x_d[P : 2 * P, 0 : 2 * FD])
        em("scalar", "dma_start", out=xh[1][:, 2 * FD :], in_=x_d[P : 2 * P, 2 * FD :])
        em("sync", "dma_start", out=xh[2], in_=x_d[2 * P : 3 * P, :])
        em("sync", "dma_start", out=xh[3], in_=x_d[3 * P : 4 * P, :])
        em("scalar", "dma_start", out=consts_s, in_=consts_d[:, :])
        em("scalar", "dma_start", out=indb_s, in_=indb_d[:, :])
        if has_bq:
            em("scalar", "dma_start", out=u_s, in_=u_d[:, :])
        for t in range(NPR):
            em("scalar", "dma_start", out=mt8[t], in_=wall_d[:, t, :, :])
        for t in range(NPR):
            em("scalar", "dma_start", out=wpv8[t], in_=wall_d[:, 2 + t, :, :])
        em("vector", "memset", eps_s, EPS)
        em("vector", "memset", ebias_s, EXP_BIAS)
        em("gpsimd", "memset", ones128, SC_W)

        # per-channel mean / E[x^2]: DVE bn_stats for t0,t2,t3 in DMA
        # arrival order; chunk t1 on ACT via accum_out (Copy -> sum(x),
        # Square -> sum(x^2)), dumping the main output into a scratch
        # tile. (Giving ACT a second chunk measured ~1.5us SLOWER: its
        # four serial 2us accum passes overtake the DVE chain.)
        sx3 = work.tile([P, 4], f32, tag="sx3", name="sx3")
        xdump = work.tile([P, H], f16, tag="xdump", name="xdump")
        for k, t in enumerate((1,)):
            em(
                "scalar",
                "activation",
                out=xdump,
                in_=xh[t],
                func=AF.Copy,
                accum_out=sx3[:, 2 * k : 2 * k + 1],
            )
            em(
                "scalar",
                "activation",
                out=xdump,
                in_=xh[t],
                func=AF.Square,
                accum_out=sx3[:, 2 * k + 1 : 2 * k + 2],
            )
        for t in (0, 2, 3):
            bn6 = work.tile([P, 4, 6], f32, tag="bn6", name="bn6")
            for sg in range(4):
                em(
                    "vector",
                    "bn_stats",
                    out=bn6[:, sg, :],
                    in_=xh[t][:, sg * FD : (sg + 1) * FD],
                )
            mv = work.tile([P, 2], f32, tag="mv", name="mv")
            em("vector", "bn_aggr", out=mv, in_=bn6)
            em("vector", "tensor_copy", out=stat[:, t : t + 1], in_=mv[:, 0:1])
            # E[x^2] = var + mean^2 in one fused op
            em(
                "vector",
                "scalar_tensor_tensor",
                out=stat[:, NCH + t : NCH + t + 1],
                in0=mv[:, 0:1],
                scalar=mv[:, 0:1],
                in1=mv[:, 1:2],
                op0=OP.mult,
                op1=OP.add,
            )
        for k, t in enumerate((1,)):
            em(
                "vector",
                "tensor_scalar_mul",
                stat[:, t : t + 1],
                sx3[:, 2 * k : 2 * k + 1],
                1.0 / H,
            )
            em(
                "vector",
                "tensor_scalar_mul",
                stat[:, NCH + t : NCH + t + 1],
                sx3[:, 2 * k + 1 : 2 * k + 2],
                1.0 / H,
            )
        # indg holds 1/16 so this directly yields group means of [m, E[x^2]]
        g_ps = ps_aux.tile([GPC, 2 * NCH], f32, tag="aux", name="gps")
        em("tensor", "matmul", g_ps, lhsT=indg_s, rhs=stat, start=True, stop=True)
        em("vector", "tensor_copy", out=stats2, in_=g_ps)
        m2g = work.tile([GPC, NCH], f32, tag="m2g", name="m2g")
        em("vector", "tensor_mul", out=m2g, in0=stats2[:, 0:NCH], in1=stats2[:, 0:NCH])
        # 1/sqrt(v+eps) as sqrt(1/(v+eps)): eps folds into the variance
        # subtraction and both DVE ops run back-to-back (one less
        # cross-engine hop on the serial a/b chain)
        var_t = work.tile([GPC, NCH], f32, tag="var", name="var")
        em(
            "vector",
            "scalar_tensor_tensor",
            out=var_t,
            in0=stats2[:, NCH :],
            scalar=float(EPS),
            in1=m2g,
            op0=OP.add,
            op1=OP.subtract,
        )
        srt = work.tile([GPC, NCH], f32, tag="srt", name="srt")
        em("vector", "reciprocal", out=srt, in_=var_t)
        em("scalar", "activation", out=stats2[:, NCH :], in_=srt, func=AF.Sqrt)
        bc_ps = ps_aux.tile([P, 2 * NCH], f32, tag="aux", name="bcps")
        em("tensor", "matmul", bc_ps, lhsT=indb_s, rhs=stats2, start=True, stop=True)
        em("vector", "tensor_mul", out=a_s, in0=bc_ps[:, NCH : 2 * NCH], in1=gamma_s)
        tmp_ma = work.tile([P, NCH], f32, tag="tmp_ma", name="tmp_ma")
        em("vector", "tensor_mul", out=tmp_ma, in0=bc_ps[:, 0:NCH], in1=a_s)
        em("vector", "tensor_sub", out=b_s, in0=beta_s, in1=tmp_ma)
        # xq8 = fp8(a*x + b) into DoubleRow pair tiles, sg-major so the z
        # GEMM can chase slice by slice; alternate DVE/ACT. Raw x stays in
        # xh for the residual.
        for sg in range(4):
            for t in range(NCH):
                sl = slice(sg * FD, (sg + 1) * FD)
                osl = xq8[t // 2][:, t % 2, sl]
                if t % 2 == 0:
                    em(
                        "vector",
                        "tensor_scalar",
                        out=osl,
                        in0=xh[t][:, sl],
                        scalar1=a_s[:, t : t + 1],
                        scalar2=b_s[:, t : t + 1],
                        op0=OP.mult,
                        op1=OP.add,
                    )
                else:
                    em(
                        "scalar",
                        "activation",
                        out=osl,
                        in_=xh[t][:, sl],
                        func=AF.Identity,
                        scale=a_s[:, t : t + 1],
                        bias=b_s[:, t : t + 1],
                    )

        # ---- phase B: z8 = fp8(16 M @ hn) (n-major to chase the quantize) ----
        ev = 0
        for n in range(NIT):
            for a in range(NCH):
                ps = ps_mm.tile([P, FD], f32, tag="mm", name="mmps")
                for t in range(NPR):
                    em(
                        "tensor",
                        "matmul",
                        ps,
                        lhsT=mt8[t][:, :, a * P : (a + 1) * P],
                        rhs=xq8[t][:, :, n * FD : (n + 1) * FD],
                        start=(t == 0),
                        stop=(t == NPR - 1),
                        perf_mode=DR,
                    )
                osl = zt8[a // 2][:, a % 2, n * FD : (n + 1) * FD]
                # all PSUM->fp8 quantize copies ride DVE: ACT is the
                # co-critical engine (64 exps saturate it through phase D)
                em("vector", "tensor_copy", out=osl, in_=ps)
                ev += 1

        # ---- phase C: vT8 = fp8(hn^T (16 WPV^T)) GEMM ----
        for j in range(NJT):
            ps = ps_mm.tile([P, FD], f32, tag="mm", name="mmps")
            for t in range(NPR):
                em(
                    "tensor",
                    "matmul",
                    ps,
                    lhsT=xq8[t][:, :, j * P : (j + 1) * P],
                    rhs=wpv8[t],
                    start=(t == 0),
                    stop=(t == NPR - 1),
                    perf_mode=DR,
                )
            osl = vT8[j // 2][:, j % 2, :]
            em("vector", "tensor_copy", out=osl, in_=ps)
            ev += 1
        if has_bq:
            # g[j] = (Wk.T bq) . hn[:, j] * SCL - 2 ln 2, added to scores
            # inside the exp (u8 carries 16 Wk.T bq in fp8).
            for j in range(NJT):
                gp = ps_aux.tile([P, 1], f32, tag="aux", name="gps1")
                for c in range(NCH):
                    em(
                        "tensor",
                        "matmul",
                        gp,
                        lhsT=xq8[c // 2][:, c % 2, j * P : (j + 1) * P],
                        rhs=u_s[:, c : c + 1],
                        start=(c == 0),
                        stop=(c == NCH - 1),
                    )
                em(
                    "vector",
                    "tensor_scalar",
                    out=g_s[:, j : j + 1],
                    in0=gp,
                    scalar1=SCL / SC_W,
                    scalar2=EXP_BIAS,
                    op0=OP.mult,
                    op1=OP.add,
                )

        # ---- phase D: attention + proj per i-tile ----
        for it in range(NIT):
            i0 = it * FD
            est8 = [
                work.tile([P, 2, FD], f8, tag=f"est{t}", name=f"est{t}")
                for t in range(NJP)
            ]
            rb_s = work.tile([P, FD], f32, tag="rb_s", name="rb_s")
            rb_ps = ps_aux.tile([P, FD], f32, tag="aux", name="rbps")
            for j in range(NJT):
                ps = ps_st.tile([P, FD], f32, tag="st", name="stps")
                for t in range(NPR):
                    em(
                        "tensor",
                        "matmul",
                        ps,
                        lhsT=xq8[t][:, :, j * P : (j + 1) * P],
                        rhs=zt8[t][:, :, i0 : i0 + FD],
                        start=(t == 0),
                        stop=(t == NPR - 1),
                        perf_mode=DR,
                    )
                em(
                    "scalar",
                    "activation",
                    out=est8[j // 2][:, j % 2, :],
                    in_=ps,
                    func=AF.Exp,
                    scale=SCL / SC_W,
                    bias=g_s[:, j : j + 1] if has_bq else ebias_s,
                )
            # denominator: the PE itself sums est8 pairs over partitions
            # via DoubleRow ones-matmuls (ones128 holds 16.0, folding
            # the vp scale) accumulated into rb_ps -- a DVE/Pool add
            # tree over the fp8 tiles is 1.4-1.9us per [128,1024] add
            # and stalled the PE 7-8us per i-tile at the den handoff.
            # Only the last ones-matmul can wait on ACT (~0.7us for the
            # final exp); the rest consume long-finished est8 pairs.
            for t in range(NJP):
                em(
                    "tensor",
                    "matmul",
                    rb_ps,
                    lhsT=ones128,
                    rhs=est8[t],
                    start=(t == 0),
                    stop=(t == NJP - 1),
                    perf_mode=DR,
                )
            # 1/(16 den) on ACT as exp(-ln(x)) (bass blocks
            # AF.Reciprocal; the ~1e-3 LUT error is far under the fp8
            # noise). DVE's reciprocal takes ~4us/[128,512] and sat on
            # the PSUM-bank recycle path, stalling the PE ~5us per
            # i-tile.
            rb_ln = work.tile([P, FD], f32, tag="rb_ln", name="rb_ln")
            em("scalar", "activation", out=rb_ln, in_=rb_ps, func=AF.Ln)
            em(
                "scalar",
                "activation",
                out=rb_s,
                in_=rb_ln,
                func=AF.Exp,
                scale=-1.0,
            )
            # out[o, i] = (sum_j 16 vp[j, o] est[j, i]) / (16 den) + bp2 + x
            # o_s chains alternate DVE/Pool so the last i-tile's serial
            # epilogue halves.
            for o in range(NCH):
                ps = ps_mm.tile([P, FD], f32, tag="mm", name="mmps")
                for t in range(NJP):
                    em(
                        "tensor",
                        "matmul",
                        ps,
                        lhsT=vT8[t][:, :, o * P : (o + 1) * P],
                        rhs=est8[t],
                        start=(t == 0),
                        stop=(t == NJP - 1),
                        perf_mode=DR,
                    )
                # both ops stay on DVE: GPSIMD cannot read PSUM (mul) and
                # TensorScalarPtr is not a Pool opcode (stt).
                o_s = work.tile([P, FD], f32, tag=f"o_s{o}", name=f"o_s{o}")
                em("vector", "tensor_mul", out=o_s, in0=ps, in1=rb_s)
                em(
                    "vector",
                    "scalar_tensor_tensor",
                    out=o_s,
                    in0=o_s,
                    scalar=bp2_s[:, o : o + 1],
                    in1=xh[o][:, i0 : i0 + FD],
                    op0=OP.add,
                    op1=OP.add,
                )
                # mid-tile outputs all ride sync (idle through phase D);
                # the last i-tile splits across sync+scalar for a fast
                # drain. gpsimd is kept DMA-free (see head comment).
                if it == NIT - 1:
                    q = "sync" if o % 2 == 0 else "scalar"
                else:
                    q = "sync"
                em(
                    q,
                    "dma_start",
                    out=out_d[o * P : (o + 1) * P, i0 : i0 + FD],
                    in_=o_s,
                )
        em.plant_tail()

    from concourse import mybir as _mybir

    deficit = redistribute_waits(nc, em, _mybir)
    return nc, em, deficit


_BUILT_MAP = {}


def get_built(has_bq=False):
    if has_bq not in _BUILT_MAP:
        needed = {}
        deficit = None
        for attempt in range(8):
            nc, em, deficit = _build_nc(dict(needed), has_bq=has_bq)
            if not deficit:
                break
            for key, n in deficit.items():
                needed[key] = max(needed.get(key, 0), n)
        else:
            raise RuntimeError(f"spare-wait fixpoint did not converge: {deficit}")
        from concourse import mybir

        bad = check_wait_budget(nc, em, mybir)
        if bad:
            raise RuntimeError(f"instructions over wait budget: {bad[:10]}")
        _BUILT_MAP[has_bq] = nc
    return _BUILT_MAP[has_bq]


def _host_prep(x, gamma, beta, wq, bq, wk, bk, wv, bv, wp, bp):
    import ml_dtypes

    f = np.float32
    f8 = ml_dtypes.float8_e4m3

    def t128(v):  # [512] -> [128, 4] with element (p, t) = v[t*128 + p]
        return np.ascontiguousarray(v.reshape(NCH, P).T.astype(f))

    ind_g = np.zeros((P, GPC), f)
    ind_g[np.arange(P), np.arange(P) // CPG] = 1.0 / CPG
    ind_b = np.zeros((GPC, P), f)
    ind_b[np.arange(P) // CPG, np.arange(P)] = 1.0
    bp2 = wp.astype(f) @ bv.astype(f) + bp.astype(f)
    # M = Wk.T @ Wq folds the q and k GEMMs into one (scores are bilinear in
    # hn); lhsT layout needs MT = M.T. bk cancels in the softmax; bq needs
    # the u-correction path (zero in this problem). 16x host scale keeps
    # the ~N(0,1/512) entries clear of the e4m3 subnormal cutoff.
    mt = (SC_W * (wq.astype(np.float64).T @ wk.astype(np.float64))).astype(f8)
    u = (wk.astype(np.float64).T @ bq.astype(np.float64)).astype(f)
    wpvT = (SC_W * (wp.astype(np.float64) @ wv.astype(np.float64)).T).astype(f8)

    def pair(w, t):  # [128, 2, 512] DoubleRow slab for chunks (2t, 2t+1)
        return np.stack(
            [w[(2 * t) * P : (2 * t + 1) * P, :], w[(2 * t + 1) * P : (2 * t + 2) * P, :]],
            axis=1,
        )

    wall = np.stack(
        [pair(mt, 0), pair(mt, 1), pair(wpvT, 0), pair(wpvT, 1)], axis=1
    )  # [P, 4, 2, FD] fp8
    consts = np.concatenate([t128(gamma), t128(beta), t128(bp2), ind_g], axis=1)
    shared = {
        "wall": np.ascontiguousarray(wall),
        "consts": np.ascontiguousarray(consts),
        "ind_b": ind_b,
        "u8": np.ascontiguousarray((SC_W * t128(u)).astype(f8)),
    }
    return [
        {"x": np.ascontiguousarray(x[b].astype(np.float16)), **shared}
        for b in range(B)
    ], bool(np.any(bq != 0))


def run(inputs, trace=False, **kw):
    from concourse.bass_utils import run_bass_kernel_spmd

    in_maps, has_bq = _host_prep(**{k: np.asarray(v) for k, v in inputs.items()})
    nc = get_built(has_bq=has_bq)
    res = run_bass_kernel_spmd(nc, in_maps, list(range(B)), trace=trace, **kw)
    out = np.stack([res.results[b]["out"] for b in range(B)]).astype(np.float32)
    return out, res


def kernel(**inputs):
    out, _ = run(inputs, trace=False)
    return out
